# revision 23
# baseline (speedup 1.0000x reference)
"""NT-Xent loss kernel for Trainium2 (8 NeuronCores, Bass/Tile).

Row-sharded + symmetric: z = concat(z1, z2) cast to bf16 on the host;
core c receives np.roll(z, -1024*c, axis=0) so every core works on
local rows [0, 1024). Using sim's symmetry, each core computes only
similarity columns [0, 5120) (blocks 0-4 of its rotated view): blocks
5/6/7 of its rows are transposes of blocks 3/2/1 of cores r+5/r+6/r+7,
whose exp column-sums are exported and merged by the host. Block 4 is
computed by both members of each (r, r+4) pair, so it needs no export.

Per core:
  1. DMA rotated z rows [0, 5120) bf16 (2 batched DMAs per 8-tile group).
  2. Row norms on DVE: batched square + 3D reduce, then 1/norm with a
     magic-seed + 2-step-Newton rsqrt (DVE only) so the scalar engine
     runs nothing but Exp -> exactly one activation-table load.
  3. Normalize to bf16, PE-transpose, copy/cast to fp8e4 znT8.
  4. Main loop per (chunk, row-tile): fp8 DoubleRow matmuls (K=256 in
     one instruction), diagonal masked to -1e30 via identity subtract,
     one ACT Exp(scale=10) per chunk with accum_out row sums; exp
     values for blocks 1-3 written (fp8) to SBUF for the colsum export.
  5. Column sums of exp(blocks 1-3) via fp8 DoubleRow ones-matmuls
     (k-dim = row-tile pairs), staged to SBUF and DMA'd out.
  6. Positive-pair sims extracted from the block-4 PSUM diagonal.

Host: merges direct row sums with imported column sums, then
loss = (sum(ln(rowsum)) - 10 * sum(pos)) / 8192 in fp64.
"""

import sys

if "/opt/trn_rl_repo" not in sys.path:
    sys.path.insert(0, "/opt/trn_rl_repo")

import numpy as np

import concourse.bacc as bacc
import concourse.mybir as mybir
import concourse.tile as tile
from concourse.masks import make_identity

P = 128
D = 256
M = 8192            # 2N rows
NCORES = 8
NT = M // P         # 64 row tiles of the full z
IT = (M // NCORES) // P   # 8 row tiles owned per core
TEMP_INV = 10.0     # 1 / temperature
F32 = mybir.dt.float32
BF16 = mybir.dt.bfloat16
FP8 = mybir.dt.float8e5
FP8E4 = mybir.dt.float8e4
CHUNK = 2048        # columns of sim handled per PSUM tile / ACT pass
NCH = M // CHUNK    # 4 col chunks per row tile
GK = 8              # row tiles per prologue group
NG = NT // GK       # 8 groups

_nc_cache = None


def _build():
    nc = bacc.Bacc(None, target_bir_lowering=False)
    z = nc.dram_tensor("z", [M, D], BF16, kind="ExternalInput")
    out = nc.dram_tensor("out", [P, 2, IT], F32, kind="ExternalOutput")
    cs = nc.dram_tensor("cs", [1, 3072], F32, kind="ExternalOutput")

    AF = mybir.ActivationFunctionType
    ALU = mybir.AluOpType
    DR = mybir.MatmulPerfMode.DoubleRow
    I32 = mybir.dt.int32

    with (
        tile.TileContext(nc) as tc,
        tc.tile_pool(name="big", bufs=1) as big,
        tc.tile_pool(name="small", bufs=1) as small,
        tc.tile_pool(name="zpool", bufs=4) as zpool,
        tc.tile_pool(name="znpool", bufs=16) as znpool,
        tc.tile_pool(name="psp", bufs=2, space="PSUM") as psp,
    ):
        znT8 = big.tile([P, 2, M], FP8E4)    # normalized z, transposed, fp8
        # Dead output buffers (data never read; only accum_out matters).
        # ACT encodes a single sync-wait per instruction, so ACT ops write
        # never-reused subtiles; same for the DVE mask-mult outs.
        exp_dead = big.tile([P, 8, CHUNK], FP8)
        # real exp values (fp8) for phases 0/1: feed colsum ones-matmuls
        expv = big.tile([P, 2, IT, CHUNK], FP8E4)
        ones8 = small.tile([P, 2, 32], FP8E4)
        nc.gpsimd.memset(ones8, 1.0)
        csb = small.tile([P, 3072], F32, partitions=1) if False else \
            big.tile([1, 3072], F32)
        sq_dead = big.tile([P, NT, D], BF16)
        md_dead = big.tile([P, IT, P], F32)
        ss = small.tile([P, NT], F32)        # row norm^2
        lnss = small.tile([P, NT], F32)      # ln(norm^2)
        rn = small.tile([P, NT], F32)        # 1 / max(norm, eps)
        identB = small.tile([P, P], BF16)    # transposes + pos extraction
        make_identity(nc, identB)
        identBig = small.tile([P, P], F32)   # 1e30 * I for diag masking
        make_identity(nc, identBig)
        nc.vector.tensor_scalar_mul(identBig, identBig, 1.0e30)

        acc = small.tile([P, IT, NCH + 1], F32)  # exp row-sum partials
        pp = small.tile([P, IT], F32)        # positive-pair sim (pre-temp)

        zv = z.rearrange("(t p) d -> p t d", p=P)

        def prologue(g):
            halves = []
            for h in range(2):
                zrt = zpool.tile([P, GK // 2, D], BF16, tag="zrt",
                                 name=f"zrt_{g}_{h}")
                (nc.sync if h == 0 else nc.gpsimd).dma_start(
                    out=zrt,
                    in_=zv[:, g * GK + h * (GK // 2) : g * GK + (h + 1) * (GK // 2), :],
                )
                halves.append(zrt)
            # batched square + row-sums: one [128,1024] mult and one 3D
            # reduce per half-group
            for h in range(2):
                t0 = g * GK + h * (GK // 2)
                nc.vector.tensor_tensor(
                    out=sq_dead[:, t0 : t0 + GK // 2, :].rearrange(
                        "p t d -> p (t d)"),
                    in0=halves[h].rearrange("p t d -> p (t d)"),
                    in1=halves[h].rearrange("p t d -> p (t d)"),
                    op=ALU.mult,
                )
                nc.vector.reduce_sum(
                    ss[:, t0 : t0 + GK // 2],
                    sq_dead[:, t0 : t0 + GK // 2, :],
                    axis=mybir.AxisListType.X,
                )
            sl = slice(g * GK, (g + 1) * GK)
            # 1/sqrt(ss) fully on DVE (magic-seed + 2 Newton steps) so the
            # scalar engine only ever runs Exp -> exactly one table load.
            ssg = ss[:, sl]
            rng_ = rn[:, sl]
            t1 = lnss[:, sl]
            si = ssg.bitcast(I32)
            yi = rng_.bitcast(I32)
            nc.vector.tensor_scalar(yi, si, 1, None, op0=ALU.arith_shift_right)
            nc.vector.tensor_scalar(yi, yi, 0xFFFFFFFF, None, op0=ALU.bitwise_xor)
            nc.vector.tensor_scalar(yi, yi, 0x5F3759DF + 1, None, op0=ALU.add)
            for _ in range(2):
                nc.vector.tensor_tensor(out=t1, in0=rng_, in1=rng_, op=ALU.mult)
                nc.vector.tensor_tensor(out=t1, in0=t1, in1=ssg, op=ALU.mult)
                nc.vector.tensor_scalar(t1, t1, -0.5, 1.5, op0=ALU.mult,
                                        op1=ALU.add)
                nc.vector.tensor_tensor(out=rng_, in0=rng_, in1=t1, op=ALU.mult)
            zn8s = []
            for j in range(GK):
                t = g * GK + j
                zn8 = znpool.tile([P, D], BF16, tag="zn8", name=f"zn8_{t}")
                nc.vector.tensor_scalar_mul(zn8, halves[j // (GK // 2)][:, j % (GK // 2), :], rn[:, t : t + 1])
                zn8s.append(zn8)
            # PE-transpose this group's 8 tiles (16 [128,128] bf16 blocks)
            pt = psp.tile([P, 2, GK, P], BF16, tag="ps", name=f"pt_{g}")
            for j in range(GK):
                for k in range(2):
                    nc.tensor.transpose(
                        pt[:, k, j, :], zn8s[j][:, k * P : (k + 1) * P], identB
                    )
            # PSUM bf16 -> SBUF fp8e4 (cast during copy); route some to the
            # scalar engine (Copy is in every act table -> no table load)
            for k in range(2):
                dst = znT8[:, k, g * (GK * P) : (g + 1) * (GK * P)]
                srcp = pt[:, k].rearrange("p j c -> p (j c)")
                if k == 1 and g % 2 == 1:
                    nc.scalar.activation(dst, srcp, AF.Copy)
                else:
                    nc.vector.tensor_copy(out=dst, in_=srcp)

        def phase(c, col0=None, width=CHUNK, slot=None, mask=False, pos=False):
            # row block x col chunk: fp8 DoubleRow matmuls + fused exp sums
            if col0 is None:
                col0 = c * CHUNK
            if slot is None:
                slot = c + 1
            for i in range(IT):
                ps = psp.tile([P, width], F32, tag="ps", name=f"ps_{c}_{i}_{slot}")
                for n in range(width // 512):
                    nc.tensor.matmul(
                        ps[:, n * 512 : (n + 1) * 512],
                        lhsT=znT8[:, :, i * P : (i + 1) * P],
                        rhs=znT8[
                            :, :, col0 + n * 512 : col0 + (n + 1) * 512
                        ],
                        start=True,
                        stop=True,
                        perf_mode=DR,
                    )
                if mask:
                    # mask self-sim: ps[p, i*128+p] -= 1e30 -> exp gives 0
                    nc.vector.tensor_tensor(
                        out=ps[:, i * P : (i + 1) * P],
                        in0=ps[:, i * P : (i + 1) * P],
                        in1=identBig,
                        op=ALU.subtract,
                    )
                if pos:
                    # positive pair: col 4096 + local row -> diagonal of the
                    # [128,128] block at col offset i*128 within this chunk
                    nc.vector.tensor_tensor(
                        out=md_dead[:, i, :],
                        in0=ps[:, i * P : (i + 1) * P],
                        in1=identB,
                        op=ALU.mult,
                    )
                    nc.vector.reduce_sum(
                        pp[:, i : i + 1], md_dead[:, i, :],
                        axis=mybir.AxisListType.X,
                    )
                nc.scalar.activation(
                    out=(expv[:, c, i, :width] if c < 2
                         else exp_dead[:, i, :width]),
                    in_=ps[:],
                    func=AF.Exp,
                    scale=TEMP_INV,
                    accum_out=acc[:, i, slot : slot + 1],
                )

        def colsums(ph, scol, ncols, csoff):
            # column sums of exp over this phase's 1024 local rows via fp8
            # DoubleRow ones-matmuls (k-dim = row-tile pairs); export to DRAM
            csp = psp.tile([P, CHUNK], F32, tag="ps", name=f"cs_{ph}")
            for s in range(ncols // 512):
                o = scol + s * 512
                for q in range(IT // 2):
                    nc.tensor.matmul(
                        csp[0:32, s * 512 : (s + 1) * 512],
                        lhsT=ones8,
                        rhs=expv[:, ph, 2 * q : 2 * q + 2, o : o + 512],
                        start=(q == 0),
                        stop=(q == IT // 2 - 1),
                        perf_mode=DR,
                    )
            nc.scalar.activation(csb[0:1, csoff : csoff + ncols],
                                 csp[0:1, 0:ncols], AF.Copy)
            nc.sync.dma_start(
                out=cs[0:1, csoff : csoff + ncols],
                in_=csb[0:1, csoff : csoff + ncols])

        # Symmetric scheme: compute sim cols [0, 5120) only (blocks 0-4).
        # Blocks 5/6/7 of this core's rows are transposes of blocks 3/2/1
        # of cores r+5/r+6/r+7, so their row-sum contributions arrive as
        # exported column sums of exp (host merges). Block 4 is computed
        # by both members of each (r, r+4) pair - no export needed.
        prologue(0)
        prologue(1)
        phase(0, slot=1, mask=True)
        prologue(2)
        prologue(3)
        phase(1, slot=2)
        colsums(0, 1024, 1024, 0)
        prologue(4)
        colsums(1, 0, 2048, 1024)
        phase(2, width=1024, slot=3, pos=True)

        # ---- tail: ship per-row exp-sums and raw positive sims; the host
        # finishes with ln / scale / mean (8192 scalars, fp64) ----
        outs = small.tile([P, 2, IT], F32)
        nc.vector.reduce_sum(outs[:, 0, :], acc, axis=mybir.AxisListType.X)
        nc.vector.tensor_copy(out=outs[:, 1, :], in_=pp)
        nc.sync.dma_start(out=out[:], in_=outs)

    nc.finalize()
    return nc


def _get_nc():
    global _nc_cache
    if _nc_cache is None:
        _nc_cache = _build()
    return _nc_cache


def _run_cores(z: np.ndarray, trace: bool = False):
    """Run the SPMD kernel on 8 cores. Returns per-core results + perf."""
    from concourse.bass_utils import run_bass_kernel_spmd

    import ml_dtypes

    nc = _get_nc()
    zb = z.astype(ml_dtypes.bfloat16)
    rows_per_core = M // NCORES
    in_maps = [
        {"z": np.ascontiguousarray(np.roll(zb, -rows_per_core * c, axis=0))}
        for c in range(NCORES)
    ]
    res = run_bass_kernel_spmd(
        nc, in_maps, core_ids=list(range(NCORES)), trace=trace
    )
    return res


def kernel(z1: np.ndarray, z2: np.ndarray) -> np.ndarray:
    z = np.concatenate(
        [np.asarray(z1, np.float32), np.asarray(z2, np.float32)], axis=0
    )
    res = _run_cores(z)
    total = np.zeros(M)
    pos_sum = 0.0
    for cc, r in enumerate(res.results):
        part = np.asarray(r["out"], np.float64)     # [P, 2, IT]
        csv = np.asarray(r["cs"], np.float64).ravel()  # [3072] cols 1024..4095
        rows = (1024 * cc + 128 * np.arange(IT)[None, :]
                + np.arange(P)[:, None]) % M        # [P, IT]
        total[rows] += part[:, 0, :]
        cols = (1024 + np.arange(3072) + 1024 * cc) % M
        np.add.at(total, cols, csv)
        pos_sum += TEMP_INV * part[:, 1, :].sum()
    lse_sum = np.log(total).sum()
    return np.float32((lse_sum - pos_sum) / M)


# revision 25
# speedup vs baseline: 1.0055x; 1.0055x over previous
"""NT-Xent loss kernel for Trainium2 (8 NeuronCores, Bass/Tile).

Row-sharded + symmetric: z = concat(z1, z2) cast to bf16 on the host;
core c receives np.roll(z, -1024*c, axis=0) so every core works on
local rows [0, 1024). Using sim's symmetry, each core computes only
similarity columns [0, 5120) (blocks 0-4 of its rotated view): blocks
5/6/7 of its rows are transposes of blocks 3/2/1 of cores r+5/r+6/r+7,
whose exp column-sums are exported and merged by the host. Block 4 is
computed by both members of each (r, r+4) pair, so it needs no export.

Per core:
  1. DMA rotated z rows [0, 5120) bf16 (2 batched DMAs per 8-tile group).
  2. Row norms on DVE: batched square + 3D reduce, then 1/norm with a
     magic-seed + 2-step-Newton rsqrt (DVE only) so the scalar engine
     runs nothing but Exp -> exactly one activation-table load.
  3. Normalize to bf16, PE-transpose, copy/cast to fp8e4 znT8.
  4. Main loop per (chunk, row-tile): fp8 DoubleRow matmuls (K=256 in
     one instruction), diagonal masked to -1e30 via identity subtract,
     one ACT Exp(scale=10) per chunk with accum_out row sums; exp
     values for blocks 1-3 written (fp8) to SBUF for the colsum export.
  5. Column sums of exp(blocks 1-3) via fp8 DoubleRow ones-matmuls
     (k-dim = row-tile pairs), staged to SBUF and DMA'd out.
  6. Positive-pair sims extracted from the block-4 PSUM diagonal.

Host: merges direct row sums with imported column sums, then
loss = (sum(ln(rowsum)) - 10 * sum(pos)) / 8192 in fp64.
"""

import sys

if "/opt/trn_rl_repo" not in sys.path:
    sys.path.insert(0, "/opt/trn_rl_repo")

import numpy as np

import concourse.bacc as bacc
import concourse.mybir as mybir
import concourse.tile as tile
from concourse.masks import make_identity

P = 128
D = 256
M = 8192            # 2N rows
NCORES = 8
NT = M // P         # 64 row tiles of the full z
IT = (M // NCORES) // P   # 8 row tiles owned per core
TEMP_INV = 10.0     # 1 / temperature
F32 = mybir.dt.float32
BF16 = mybir.dt.bfloat16
FP8 = mybir.dt.float8e5
FP8E4 = mybir.dt.float8e4
CHUNK = 2048        # columns of sim handled per PSUM tile / ACT pass
NCH = M // CHUNK    # 4 col chunks per row tile
GK = 8              # row tiles per prologue group
NG = NT // GK       # 8 groups

_nc_cache = None


def _build():
    nc = bacc.Bacc(None, target_bir_lowering=False)
    z = nc.dram_tensor("z", [M, D], BF16, kind="ExternalInput")
    out = nc.dram_tensor("out", [P, 2, IT], F32, kind="ExternalOutput")
    cs = nc.dram_tensor("cs", [1, 3072], F32, kind="ExternalOutput")

    AF = mybir.ActivationFunctionType
    ALU = mybir.AluOpType
    DR = mybir.MatmulPerfMode.DoubleRow
    I32 = mybir.dt.int32

    with (
        tile.TileContext(nc) as tc,
        tc.tile_pool(name="big", bufs=1) as big,
        tc.tile_pool(name="small", bufs=1) as small,
        tc.tile_pool(name="zpool", bufs=4) as zpool,
        tc.tile_pool(name="znpool", bufs=16) as znpool,
        tc.tile_pool(name="psp", bufs=2, space="PSUM") as psp,
    ):
        znT8 = big.tile([P, 2, M], FP8E4)    # normalized z, transposed, fp8
        # Dead output buffers (data never read; only accum_out matters).
        # ACT encodes a single sync-wait per instruction, so ACT ops write
        # never-reused subtiles; same for the DVE mask-mult outs.
        exp_dead = big.tile([P, 8, CHUNK], FP8)
        # real exp values (fp8) for phases 0/1: feed colsum ones-matmuls
        expv = big.tile([P, 2, IT, CHUNK], FP8E4)
        ones8 = small.tile([P, 2, 32], FP8E4)
        nc.gpsimd.memset(ones8, 1.0)
        csb = small.tile([P, 3072], F32, partitions=1) if False else \
            big.tile([1, 3072], F32)
        sq_dead = big.tile([P, NT, D], BF16)
        md_dead = big.tile([P, IT, P], F32)
        ss = small.tile([P, NT], F32)        # row norm^2
        lnss = small.tile([P, NT], F32)      # ln(norm^2)
        rn = small.tile([P, NT], F32)        # 1 / max(norm, eps)
        identB = small.tile([P, P], BF16)    # transposes + pos extraction
        make_identity(nc, identB)
        identBig = small.tile([P, P], F32)   # 1e30 * I for diag masking
        make_identity(nc, identBig)
        nc.vector.tensor_scalar_mul(identBig, identBig, 1.0e30)

        acc = small.tile([P, IT, NCH + 1], F32)  # exp row-sum partials
        pp = small.tile([P, IT], F32)        # positive-pair sim (pre-temp)

        zv = z.rearrange("(t p) d -> p t d", p=P)

        def prologue(g):
            halves = []
            for h in range(2):
                zrt = zpool.tile([P, GK // 2, D], BF16, tag="zrt",
                                 name=f"zrt_{g}_{h}")
                (nc.sync if h == 0 else nc.gpsimd).dma_start(
                    out=zrt,
                    in_=zv[:, g * GK + h * (GK // 2) : g * GK + (h + 1) * (GK // 2), :],
                )
                halves.append(zrt)
            # batched square + row-sums: one [128,1024] mult and one 3D
            # reduce per half-group
            for h in range(2):
                t0 = g * GK + h * (GK // 2)
                nc.vector.tensor_tensor(
                    out=sq_dead[:, t0 : t0 + GK // 2, :].rearrange(
                        "p t d -> p (t d)"),
                    in0=halves[h].rearrange("p t d -> p (t d)"),
                    in1=halves[h].rearrange("p t d -> p (t d)"),
                    op=ALU.mult,
                )
                nc.vector.reduce_sum(
                    ss[:, t0 : t0 + GK // 2],
                    sq_dead[:, t0 : t0 + GK // 2, :],
                    axis=mybir.AxisListType.X,
                )
            sl = slice(g * GK, (g + 1) * GK)
            # 1/sqrt(ss) fully on DVE (magic-seed + 2 Newton steps) so the
            # scalar engine only ever runs Exp -> exactly one table load.
            ssg = ss[:, sl]
            rng_ = rn[:, sl]
            t1 = lnss[:, sl]
            si = ssg.bitcast(I32)
            yi = rng_.bitcast(I32)
            nc.vector.tensor_scalar(yi, si, 1, None, op0=ALU.arith_shift_right)
            nc.vector.tensor_scalar(yi, yi, 0xFFFFFFFF, None, op0=ALU.bitwise_xor)
            nc.vector.tensor_scalar(yi, yi, 0x5F3759DF + 1, None, op0=ALU.add)
            for _ in range(2):
                nc.vector.tensor_tensor(out=t1, in0=rng_, in1=rng_, op=ALU.mult)
                nc.vector.tensor_tensor(out=t1, in0=t1, in1=ssg, op=ALU.mult)
                nc.vector.tensor_scalar(t1, t1, -0.5, 1.5, op0=ALU.mult,
                                        op1=ALU.add)
                nc.vector.tensor_tensor(out=rng_, in0=rng_, in1=t1, op=ALU.mult)
            zn8s = []
            for j in range(GK):
                t = g * GK + j
                zn8 = znpool.tile([P, D], BF16, tag="zn8", name=f"zn8_{t}")
                nc.vector.tensor_scalar_mul(zn8, halves[j // (GK // 2)][:, j % (GK // 2), :], rn[:, t : t + 1])
                zn8s.append(zn8)
            # PE-transpose this group's 8 tiles (16 [128,128] bf16 blocks)
            pt = psp.tile([P, 2, GK, P], BF16, tag="ps", name=f"pt_{g}")
            for j in range(GK):
                for k in range(2):
                    nc.tensor.transpose(
                        pt[:, k, j, :], zn8s[j][:, k * P : (k + 1) * P], identB
                    )
            # PSUM bf16 -> SBUF fp8e4 (cast during copy); route some to the
            # scalar engine (Copy is in every act table -> no table load)
            for k in range(2):
                dst = znT8[:, k, g * (GK * P) : (g + 1) * (GK * P)]
                srcp = pt[:, k].rearrange("p j c -> p (j c)")
                if k == 1 and g % 2 == 1:
                    nc.scalar.activation(dst, srcp, AF.Copy)
                else:
                    nc.vector.tensor_copy(out=dst, in_=srcp)

        def phase(c, col0=None, width=CHUNK, slot=None, mask=False, pos=False):
            # row block x col chunk: fp8 DoubleRow matmuls + fused exp sums
            if col0 is None:
                col0 = c * CHUNK
            if slot is None:
                slot = c + 1
            for i in range(IT):
                ps = psp.tile([P, width], F32, tag="ps", name=f"ps_{c}_{i}_{slot}")
                for n in range(width // 512):
                    nc.tensor.matmul(
                        ps[:, n * 512 : (n + 1) * 512],
                        lhsT=znT8[:, :, i * P : (i + 1) * P],
                        rhs=znT8[
                            :, :, col0 + n * 512 : col0 + (n + 1) * 512
                        ],
                        start=True,
                        stop=True,
                        perf_mode=DR,
                    )
                if mask:
                    # mask self-sim: ps[p, i*128+p] -= 1e30 -> exp gives 0
                    nc.vector.tensor_tensor(
                        out=ps[:, i * P : (i + 1) * P],
                        in0=ps[:, i * P : (i + 1) * P],
                        in1=identBig,
                        op=ALU.subtract,
                    )
                if pos:
                    # positive pair: col 4096 + local row -> diagonal of the
                    # [128,128] block at col offset i*128 within this chunk
                    nc.vector.tensor_tensor(
                        out=md_dead[:, i, :],
                        in0=ps[:, i * P : (i + 1) * P],
                        in1=identB,
                        op=ALU.mult,
                    )
                    nc.vector.reduce_sum(
                        pp[:, i : i + 1], md_dead[:, i, :],
                        axis=mybir.AxisListType.X,
                    )
                nc.scalar.activation(
                    out=(expv[:, c, i, :width] if c < 2
                         else exp_dead[:, i, :width]),
                    in_=ps[:],
                    func=AF.Exp,
                    scale=TEMP_INV,
                    accum_out=acc[:, i, slot : slot + 1],
                )

        def colsums(ph, scol, ncols, csoff):
            # column sums of exp over this phase's 1024 local rows via fp8
            # DoubleRow ones-matmuls (k-dim = row-tile pairs); export to DRAM
            csp = psp.tile([P, CHUNK], F32, tag="ps", name=f"cs_{ph}")
            for s in range(ncols // 512):
                o = scol + s * 512
                for q in range(IT // 2):
                    nc.tensor.matmul(
                        csp[0:32, s * 512 : (s + 1) * 512],
                        lhsT=ones8,
                        rhs=expv[:, ph, 2 * q : 2 * q + 2, o : o + 512],
                        start=(q == 0),
                        stop=(q == IT // 2 - 1),
                        perf_mode=DR,
                    )
            nc.scalar.activation(csb[0:1, csoff : csoff + ncols],
                                 csp[0:1, 0:ncols], AF.Copy)
            nc.sync.dma_start(
                out=cs[0:1, csoff : csoff + ncols],
                in_=csb[0:1, csoff : csoff + ncols])

        # Symmetric scheme: compute sim cols [0, 5120) only (blocks 0-4).
        # Blocks 5/6/7 of this core's rows are transposes of blocks 3/2/1
        # of cores r+5/r+6/r+7, so their row-sum contributions arrive as
        # exported column sums of exp (host merges). Block 4 is computed
        # by both members of each (r, r+4) pair - no export needed.
        prologue(0)
        prologue(1)
        phase(0, slot=1, mask=True)
        prologue(2)
        prologue(3)
        phase(1, slot=2)
        colsums(0, 1024, 1024, 0)
        prologue(4)
        colsums(1, 0, 2048, 1024)
        phase(2, width=1024, slot=3, pos=True)

        # ---- tail: ship per-row exp-sums and raw positive sims; the host
        # finishes with ln / scale / mean (8192 scalars, fp64) ----
        outs = small.tile([P, 2, IT], F32)
        nc.vector.reduce_sum(outs[:, 0, :], acc, axis=mybir.AxisListType.X)
        nc.vector.tensor_copy(out=outs[:, 1, :], in_=pp)
        nc.sync.dma_start(out=out[:], in_=outs)

    nc.finalize()
    return nc


def _get_nc():
    global _nc_cache
    if _nc_cache is None:
        _nc_cache = _build()
    return _nc_cache


def _run_cores(z: np.ndarray, trace: bool = False):
    """Run the SPMD kernel on 8 cores. Returns per-core results + perf."""
    from concourse.bass_utils import run_bass_kernel_spmd

    import ml_dtypes

    nc = _get_nc()
    zb = z.astype(ml_dtypes.bfloat16)
    rows_per_core = M // NCORES
    in_maps = [
        {"z": np.ascontiguousarray(np.roll(zb, -rows_per_core * c, axis=0))}
        for c in range(NCORES)
    ]
    res = run_bass_kernel_spmd(
        nc, in_maps, core_ids=list(range(NCORES)), trace=trace
    )
    return res


def kernel(z1: np.ndarray, z2: np.ndarray) -> np.ndarray:
    z = np.concatenate(
        [np.asarray(z1, np.float32), np.asarray(z2, np.float32)], axis=0
    )
    res = _run_cores(z)
    total = np.zeros(M)
    pos_sum = 0.0
    for cc, r in enumerate(res.results):
        part = np.asarray(r["out"], np.float64)     # [P, 2, IT]
        csv = np.asarray(r["cs"], np.float64).ravel()  # [3072] cols 1024..4095
        rows = (1024 * cc + 128 * np.arange(IT)[None, :]
                + np.arange(P)[:, None]) % M        # [P, IT]
        total[rows] += part[:, 0, :]
        cols = (1024 + np.arange(3072) + 1024 * cc) % M
        np.add.at(total, cols, csv)
        pos_sum += TEMP_INV * part[:, 1, :].sum()
    lse_sum = np.log(total).sum()
    return np.float32((lse_sum - pos_sum) / M)


# revision 26
# speedup vs baseline: 1.0332x; 1.0275x over previous
"""NT-Xent loss kernel for Trainium2 (8 NeuronCores, Bass/Tile).

Row-sharded + symmetric: z = concat(z1, z2) cast to bf16 on the host;
core c receives np.roll(z, -1024*c, axis=0) so every core works on
local rows [0, 1024). Using sim's symmetry, each core computes only
similarity columns [0, 5120) (blocks 0-4 of its rotated view): blocks
5/6/7 of its rows are transposes of blocks 3/2/1 of cores r+5/r+6/r+7,
whose exp column-sums are exported and merged by the host. Block 4 is
computed by both members of each (r, r+4) pair, so it needs no export.

Per core:
  1. DMA rotated z rows [0, 5120) bf16 (2 batched DMAs per 8-tile group).
  2. Row norms on DVE: batched square + 3D reduce, then 1/norm with a
     magic-seed + 2-step-Newton rsqrt (DVE only) so the scalar engine
     runs nothing but Exp -> exactly one activation-table load.
  3. Normalize to bf16, PE-transpose, copy/cast to fp8e4 znT8.
  4. Main loop per (chunk, row-tile): fp8 DoubleRow matmuls (K=256 in
     one instruction), diagonal masked to -1e30 via identity subtract,
     one ACT Exp(scale=10) per chunk with accum_out row sums; exp
     values for blocks 1-3 written (fp8) to SBUF for the colsum export.
  5. Column sums of exp(blocks 1-3) via fp8 DoubleRow ones-matmuls
     (k-dim = row-tile pairs), staged to SBUF and DMA'd out.
  6. Positive-pair sims extracted from the block-4 PSUM diagonal.

Host: merges direct row sums with imported column sums, then
loss = (sum(ln(rowsum)) - 10 * sum(pos)) / 8192 in fp64.
"""

import sys

if "/opt/trn_rl_repo" not in sys.path:
    sys.path.insert(0, "/opt/trn_rl_repo")

import numpy as np

import concourse.bacc as bacc
import concourse.mybir as mybir
import concourse.tile as tile
from concourse.masks import make_identity

P = 128
D = 256
M = 8192            # 2N rows
NCORES = 8
NT = M // P         # 64 row tiles of the full z
IT = (M // NCORES) // P   # 8 row tiles owned per core
TEMP_INV = 10.0     # 1 / temperature
F32 = mybir.dt.float32
BF16 = mybir.dt.bfloat16
FP8 = mybir.dt.float8e5
FP8E4 = mybir.dt.float8e4
CHUNK = 2048        # columns of sim handled per PSUM tile / ACT pass
NCH = M // CHUNK    # 4 col chunks per row tile
GK = 8              # row tiles per prologue group
NG = NT // GK       # 8 groups

_nc_cache = None


def _build():
    nc = bacc.Bacc(None, target_bir_lowering=False)
    z = nc.dram_tensor("z", [M, D], BF16, kind="ExternalInput")
    out = nc.dram_tensor("out", [P, 2, IT], F32, kind="ExternalOutput")
    cs = nc.dram_tensor("cs", [1, 3072], F32, kind="ExternalOutput")

    AF = mybir.ActivationFunctionType
    ALU = mybir.AluOpType
    DR = mybir.MatmulPerfMode.DoubleRow
    I32 = mybir.dt.int32

    with (
        tile.TileContext(nc) as tc,
        tc.tile_pool(name="big", bufs=1) as big,
        tc.tile_pool(name="small", bufs=1) as small,
        tc.tile_pool(name="zpool", bufs=4) as zpool,
        tc.tile_pool(name="znpool", bufs=16) as znpool,
        tc.tile_pool(name="psp", bufs=2, space="PSUM") as psp,
    ):
        znT8 = big.tile([P, 2, M], FP8E4)    # normalized z, transposed, fp8
        # Dead output buffers (data never read; only accum_out matters).
        # ACT encodes a single sync-wait per instruction, so ACT ops write
        # never-reused subtiles; same for the DVE mask-mult outs.
        exp_dead = big.tile([P, 8, CHUNK], FP8)
        # real exp values (fp8) for phases 0/1: feed colsum ones-matmuls
        expv = big.tile([P, 2, IT, CHUNK], FP8E4)
        ones8 = small.tile([P, 2, 32], FP8E4)
        nc.gpsimd.memset(ones8, 1.0)
        csb = small.tile([P, 3072], F32, partitions=1) if False else \
            big.tile([1, 3072], F32)
        sq_dead = big.tile([P, NT, D], BF16)
        md_dead = big.tile([P, IT, P], F32)
        ss = small.tile([P, NT], F32)        # row norm^2
        lnss = small.tile([P, NT], F32)      # ln(norm^2)
        rn = small.tile([P, NT], F32)        # 1 / max(norm, eps)
        identB = small.tile([P, P], BF16)    # transposes + pos extraction
        make_identity(nc, identB)
        identBig = small.tile([P, P], F32)   # 1e30 * I for diag masking
        make_identity(nc, identBig)
        nc.vector.tensor_scalar_mul(identBig, identBig, 1.0e30)

        acc = small.tile([P, IT, NCH + 1], F32)  # exp row-sum partials
        pp = small.tile([P, IT], F32)        # positive-pair sim (pre-temp)

        zv = z.rearrange("(t p) d -> p t d", p=P)

        def prologue(g):
            halves = []
            for h in range(2):
                zrt = zpool.tile([P, GK // 2, D], BF16, tag="zrt",
                                 name=f"zrt_{g}_{h}")
                (nc.sync if h == 0 else nc.gpsimd).dma_start(
                    out=zrt,
                    in_=zv[:, g * GK + h * (GK // 2) : g * GK + (h + 1) * (GK // 2), :],
                )
                halves.append(zrt)
            # batched square + row-sums: one [128,1024] mult and one 3D
            # reduce per half-group
            for h in range(2):
                t0 = g * GK + h * (GK // 2)
                nc.vector.tensor_tensor(
                    out=sq_dead[:, t0 : t0 + GK // 2, :].rearrange(
                        "p t d -> p (t d)"),
                    in0=halves[h].rearrange("p t d -> p (t d)"),
                    in1=halves[h].rearrange("p t d -> p (t d)"),
                    op=ALU.mult,
                )
                nc.vector.reduce_sum(
                    ss[:, t0 : t0 + GK // 2],
                    sq_dead[:, t0 : t0 + GK // 2, :],
                    axis=mybir.AxisListType.X,
                )
            sl = slice(g * GK, (g + 1) * GK)
            # 1/sqrt(ss) fully on DVE (magic-seed + 2 Newton steps) so the
            # scalar engine only ever runs Exp -> exactly one table load.
            ssg = ss[:, sl]
            rng_ = rn[:, sl]
            t1 = lnss[:, sl]
            si = ssg.bitcast(I32)
            yi = rng_.bitcast(I32)
            nc.vector.tensor_scalar(yi, si, 1, None, op0=ALU.arith_shift_right)
            nc.vector.tensor_scalar(yi, yi, 0xFFFFFFFF, None, op0=ALU.bitwise_xor)
            nc.vector.tensor_scalar(yi, yi, 0x5F3759DF + 1, None, op0=ALU.add)
            for _ in range(1):
                nc.vector.tensor_tensor(out=t1, in0=rng_, in1=rng_, op=ALU.mult)
                nc.vector.tensor_tensor(out=t1, in0=t1, in1=ssg, op=ALU.mult)
                nc.vector.tensor_scalar(t1, t1, -0.5, 1.5, op0=ALU.mult,
                                        op1=ALU.add)
                nc.vector.tensor_tensor(out=rng_, in0=rng_, in1=t1, op=ALU.mult)
            zn8s = []
            for j in range(GK):
                t = g * GK + j
                zn8 = znpool.tile([P, D], BF16, tag="zn8", name=f"zn8_{t}")
                nc.vector.tensor_scalar_mul(zn8, halves[j // (GK // 2)][:, j % (GK // 2), :], rn[:, t : t + 1])
                zn8s.append(zn8)
            # PE-transpose this group's 8 tiles (16 [128,128] bf16 blocks)
            pt = psp.tile([P, 2, GK, P], BF16, tag="ps", name=f"pt_{g}")
            for j in range(GK):
                for k in range(2):
                    nc.tensor.transpose(
                        pt[:, k, j, :], zn8s[j][:, k * P : (k + 1) * P], identB
                    )
            # PSUM bf16 -> SBUF fp8e4 (cast during copy); route some to the
            # scalar engine (Copy is in every act table -> no table load)
            for k in range(2):
                dst = znT8[:, k, g * (GK * P) : (g + 1) * (GK * P)]
                srcp = pt[:, k].rearrange("p j c -> p (j c)")
                if k == 1 and g % 2 == 1:
                    nc.scalar.activation(dst, srcp, AF.Copy)
                else:
                    nc.vector.tensor_copy(out=dst, in_=srcp)

        def phase(c, col0=None, width=CHUNK, slot=None, mask=False, pos=False):
            # row block x col chunk: fp8 DoubleRow matmuls + fused exp sums
            if col0 is None:
                col0 = c * CHUNK
            if slot is None:
                slot = c + 1
            for i in range(IT):
                ps = psp.tile([P, width], F32, tag="ps", name=f"ps_{c}_{i}_{slot}")
                for n in range(width // 512):
                    nc.tensor.matmul(
                        ps[:, n * 512 : (n + 1) * 512],
                        lhsT=znT8[:, :, i * P : (i + 1) * P],
                        rhs=znT8[
                            :, :, col0 + n * 512 : col0 + (n + 1) * 512
                        ],
                        start=True,
                        stop=True,
                        perf_mode=DR,
                    )
                if mask:
                    # mask self-sim: ps[p, i*128+p] -= 1e30 -> exp gives 0
                    nc.vector.tensor_tensor(
                        out=ps[:, i * P : (i + 1) * P],
                        in0=ps[:, i * P : (i + 1) * P],
                        in1=identBig,
                        op=ALU.subtract,
                    )
                if pos:
                    # positive pair: col 4096 + local row -> diagonal of the
                    # [128,128] block at col offset i*128 within this chunk
                    nc.vector.tensor_tensor(
                        out=md_dead[:, i, :],
                        in0=ps[:, i * P : (i + 1) * P],
                        in1=identB,
                        op=ALU.mult,
                    )
                    nc.vector.reduce_sum(
                        pp[:, i : i + 1], md_dead[:, i, :],
                        axis=mybir.AxisListType.X,
                    )
                nc.scalar.activation(
                    out=(expv[:, c, i, :width] if c < 2
                         else exp_dead[:, i, :width]),
                    in_=ps[:],
                    func=AF.Exp,
                    scale=TEMP_INV,
                    accum_out=acc[:, i, slot : slot + 1],
                )

        def colsums(ph, scol, ncols, csoff):
            # column sums of exp over this phase's 1024 local rows via fp8
            # DoubleRow ones-matmuls (k-dim = row-tile pairs); export to DRAM
            csp = psp.tile([P, CHUNK], F32, tag="ps", name=f"cs_{ph}")
            for s in range(ncols // 512):
                o = scol + s * 512
                for q in range(IT // 2):
                    nc.tensor.matmul(
                        csp[0:32, s * 512 : (s + 1) * 512],
                        lhsT=ones8,
                        rhs=expv[:, ph, 2 * q : 2 * q + 2, o : o + 512],
                        start=(q == 0),
                        stop=(q == IT // 2 - 1),
                        perf_mode=DR,
                    )
            nc.scalar.activation(csb[0:1, csoff : csoff + ncols],
                                 csp[0:1, 0:ncols], AF.Copy)
            nc.sync.dma_start(
                out=cs[0:1, csoff : csoff + ncols],
                in_=csb[0:1, csoff : csoff + ncols])

        # Symmetric scheme: compute sim cols [0, 5120) only (blocks 0-4).
        # Blocks 5/6/7 of this core's rows are transposes of blocks 3/2/1
        # of cores r+5/r+6/r+7, so their row-sum contributions arrive as
        # exported column sums of exp (host merges). Block 4 is computed
        # by both members of each (r, r+4) pair - no export needed.
        prologue(0)
        prologue(1)
        phase(0, slot=1, mask=True)
        prologue(2)
        prologue(3)
        phase(1, slot=2)
        colsums(0, 1024, 1024, 0)
        prologue(4)
        colsums(1, 0, 2048, 1024)
        phase(2, width=1024, slot=3, pos=True)

        # ---- tail: ship per-row exp-sums and raw positive sims; the host
        # finishes with ln / scale / mean (8192 scalars, fp64) ----
        outs = small.tile([P, 2, IT], F32)
        nc.vector.reduce_sum(outs[:, 0, :], acc, axis=mybir.AxisListType.X)
        nc.vector.tensor_copy(out=outs[:, 1, :], in_=pp)
        nc.sync.dma_start(out=out[:], in_=outs)

    nc.finalize()
    return nc


def _get_nc():
    global _nc_cache
    if _nc_cache is None:
        _nc_cache = _build()
    return _nc_cache


def _run_cores(z: np.ndarray, trace: bool = False):
    """Run the SPMD kernel on 8 cores. Returns per-core results + perf."""
    from concourse.bass_utils import run_bass_kernel_spmd

    import ml_dtypes

    nc = _get_nc()
    zb = z.astype(ml_dtypes.bfloat16)
    rows_per_core = M // NCORES
    in_maps = [
        {"z": np.ascontiguousarray(np.roll(zb, -rows_per_core * c, axis=0))}
        for c in range(NCORES)
    ]
    res = run_bass_kernel_spmd(
        nc, in_maps, core_ids=list(range(NCORES)), trace=trace
    )
    return res


def kernel(z1: np.ndarray, z2: np.ndarray) -> np.ndarray:
    z = np.concatenate(
        [np.asarray(z1, np.float32), np.asarray(z2, np.float32)], axis=0
    )
    res = _run_cores(z)
    total = np.zeros(M)
    pos_sum = 0.0
    for cc, r in enumerate(res.results):
        part = np.asarray(r["out"], np.float64)     # [P, 2, IT]
        csv = np.asarray(r["cs"], np.float64).ravel()  # [3072] cols 1024..4095
        rows = (1024 * cc + 128 * np.arange(IT)[None, :]
                + np.arange(P)[:, None]) % M        # [P, IT]
        total[rows] += part[:, 0, :]
        cols = (1024 + np.arange(3072) + 1024 * cc) % M
        np.add.at(total, cols, csv)
        pos_sum += TEMP_INV * part[:, 1, :].sum()
    lse_sum = np.log(total).sum()
    return np.float32((lse_sum - pos_sum) / M)
